# revision 1
# baseline (speedup 1.0000x reference)
"""DHPF kernel for Trainium2: batch-parallel 2D FFT high-pass filter.

Per NeuronCore (8 cores, one batch element each):
  f^T = F @ (F @ X)^T per channel (DFT-matrix matmuls in float32r + PE transpose),
  data-dependent cutoff search on channel 8 (energy box sums via selection-matrix
  matmuls), central-box mask, inverse transform, |.|, DMA out.
"""

import numpy as np
from contextlib import ExitStack

import jax
import concourse.bass as bass
import concourse.bacc as bacc
import concourse.mybir as mybir
from concourse.tile import TileContext

P = 128
NT = 4  # 512 / 128
H = W = 512
C = 16
B = 8
NCORES = 8
ENERGY = 0.9

F32 = mybir.dt.float32
F32R = mybir.dt.float32r
ALU = mybir.AluOpType
ACTF = mybir.ActivationFunctionType
AX = mybir.AxisListType


# ----------------------------------------------------------------- host consts
def _host_constants():
    n = np.arange(H, dtype=np.float64)
    ph = -2.0 * np.pi * np.outer(n, n) / H
    Fc = np.exp(1j * ph)  # DFT matrix
    Jc = np.conj(Fc) / H  # IDFT matrix (one axis)

    consts = {}

    ident = np.eye(P, dtype=np.float32)
    consts["ident"] = ident

    # CT(4,128) stage matrices. Stored freq order: stored row 128*k1+k2 <-> freq k1+4*k2.
    n2 = np.arange(P, dtype=np.float64)
    k2 = np.arange(P, dtype=np.float64)
    fwd_r, fwd_i, inv_r, inv_i = [], [], [], []
    for k1 in range(NT):
        # FWD lhsT[n2, k2] = exp(-2j pi n2 (k1 + 4 k2)/512)
        phF = -2.0 * np.pi * np.outer(n2, k1 + 4.0 * k2) / H
        Mf = np.exp(1j * phF)
        fwd_r.append(Mf.real)
        fwd_i.append(Mf.imag)
        # INV lhsT[kk2, nn2] = (1/512) exp(+2j pi nn2 (k1 + 4 kk2)/512)
        phI = 2.0 * np.pi * np.outer(k1 + 4.0 * k2, n2) / H  # [k2 (part), n2 (free)]
        Mi_ = np.exp(1j * phI) / H
        inv_r.append(Mi_.real)
        inv_i.append(Mi_.imag)
    consts["FWDr"] = np.concatenate(fwd_r, axis=1).astype(np.float32)
    consts["FWDi"] = np.concatenate(fwd_i, axis=1).astype(np.float32)
    consts["FWDrn"] = -consts["FWDr"]
    consts["FWDin"] = -consts["FWDi"]
    consts["INVr"] = np.concatenate(inv_r, axis=1).astype(np.float32)
    consts["INVi"] = np.concatenate(inv_i, axis=1).astype(np.float32)
    consts["INVrn"] = -consts["INVr"]
    consts["INVin"] = -consts["INVi"]

    freqmap = (np.arange(H) // P) + 4 * (np.arange(H) % P)  # stored idx -> freq

    # Asel[r, cidx] = 1 if row r in R(c=cidx+1) = [0,c) u [512-c,512); col 255 = all ones
    Asel = np.zeros((H, 256), dtype=np.float32)
    for cidx in range(255):
        c = cidx + 1
        Asel[:c, cidx] = 1.0
        Asel[H - c :, cidx] = 1.0
    Asel[:, 255] = 1.0
    consts["Asel"] = Asel[freqmap]

    Bsel = np.zeros((256, W), dtype=np.float32)
    for cidx in range(255):
        c = cidx + 1
        Bsel[cidx, :c] = 1.0
        Bsel[cidx, W - c :] = 1.0
    Bsel[255, :] = 1.0
    consts["Bsel"] = Bsel[:, freqmap]

    iota_p = np.zeros((P, NT), dtype=np.float32)
    for t in range(NT):
        iota_p[:, t] = t + 4.0 * np.arange(P)
    consts["iota_p"] = iota_p

    consts["iota_f"] = np.broadcast_to(
        freqmap.astype(np.float32), (P, W)
    ).copy()
    consts["ones_col"] = np.ones((P, 1), dtype=np.float32)
    consts["ones_row"] = np.ones((1, P), dtype=np.float32)
    return consts


CONST_DTYPES = {
    "FWDr": F32R, "FWDi": F32R, "FWDrn": F32R, "FWDin": F32R,
    "INVr": F32R, "INVi": F32R, "INVrn": F32R, "INVin": F32R,
    "ident": F32R, "Asel": F32R,
    "Bsel": F32, "iota_p": F32, "iota_f": F32, "ones_col": F32, "ones_row": F32,
}


# ----------------------------------------------------------------- kernel body
def _chunk(mat, kt, mt, blk=512):
    """lhsT chunk of a [512,512] matrix stored [128, 4*blk] in (t p) layout."""
    return mat[:, kt * blk + mt * P : kt * blk + mt * P + P]


def _blk(mat, t, blk=512):
    return mat[:, t * blk : (t + 1) * blk]


def build_nc():
    nc = bacc.Bacc()
    x_d = nc.declare_dram_parameter("x", [C, H, W], F32, isOutput=False)
    cd = {}
    shapes = {
        "FWDr": [P, W], "FWDi": [P, W], "FWDrn": [P, W], "FWDin": [P, W],
        "INVr": [P, W], "INVi": [P, W], "INVrn": [P, W], "INVin": [P, W],
        "ident": [P, P], "Asel": [H, 256], "Bsel": [256, W],
        "iota_p": [P, NT], "iota_f": [P, W], "ones_col": [P, 1],
        "ones_row": [1, P],
    }
    for name, shp in shapes.items():
        cd[name] = nc.declare_dram_parameter(name, shp, CONST_DTYPES[name], isOutput=False)
    out_d = nc.declare_dram_parameter("out", [C, H, W], F32, isOutput=True)

    with ExitStack() as ctx:
        tc = ctx.enter_context(TileContext(nc))
        cpool = ctx.enter_context(tc.tile_pool(name="consts", bufs=1))
        persist = ctx.enter_context(tc.tile_pool(name="persist", bufs=1))
        work = ctx.enter_context(tc.tile_pool(name="work", bufs=1))
        xpool = ctx.enter_context(tc.tile_pool(name="xp", bufs=2))
        opool = ctx.enter_context(tc.tile_pool(name="op", bufs=1))
        psmm = ctx.enter_context(tc.tile_pool(name="psmm", bufs=4, space="PSUM"))
        pstp = ctx.enter_context(tc.tile_pool(name="pstp", bufs=2, space="PSUM"))
        pssm = ctx.enter_context(tc.tile_pool(name="pssm", bufs=2, space="PSUM"))

        # ---- load constants
        cs = {}
        t = cpool.tile([P, NT * 256], F32R, tag="Asel")
        nc.gpsimd.dma_start(
            t[:].rearrange("p (t j) -> p t j", t=NT),
            cd["Asel"].ap().rearrange("(t p) j -> p t j", p=P),
        )
        cs["Asel"] = t
        t = cpool.tile([P, 2 * W], F32, tag="Bsel")
        nc.gpsimd.dma_start(
            t[:].rearrange("p (t j) -> p t j", t=2),
            cd["Bsel"].ap().rearrange("(t p) j -> p t j", p=P),
        )
        cs["Bsel"] = t
        for name in ("FWDr", "FWDi", "FWDrn", "FWDin", "INVr", "INVi", "INVrn", "INVin"):
            t = cpool.tile([P, W], F32R, tag=name)
            nc.gpsimd.dma_start(t[:], cd[name].ap())
            cs[name] = t
        for name, shp in (("ident", [P, P]), ("iota_p", [P, NT]),
                          ("iota_f", [P, W]), ("ones_col", [P, 1])):
            t = cpool.tile(shp, CONST_DTYPES[name], tag=name)
            nc.gpsimd.dma_start(t[:], cd[name].ap())
            cs[name] = t
        t = cpool.tile([1, P], F32, tag="ones_row")
        nc.gpsimd.dma_start(t[:], cd["ones_row"].ap())
        cs["ones_row"] = t

        # ---- helpers ------------------------------------------------------
        def load_x(ch):
            xt = xpool.tile([P, NT * W], F32R, tag="x")
            nc.gpsimd.dma_start(
                xt[:].rearrange("p (t j) -> p t j", t=NT),
                x_d.ap()[ch].rearrange("(t p) j -> p t j", p=P),
            )
            return xt

        _evac_rr = [0]

        def evac_copy(dst, src):
            # alternate DVE/ACT so neither engine becomes the copy bottleneck
            _evac_rr[0] ^= 1
            if _evac_rr[0]:
                nc.vector.tensor_copy(dst, src)
            else:
                nc.scalar.copy(dst, src)

        _tt_rr = [0]

        def tt_op(dst, a, b, op):
            # DVE only: GpSimd elementwise locks the shared DVE SBUF port pair
            # and contends with SWDGE descriptor generation (measured slower).
            nc.vector.tensor_tensor(dst, a, b, op)

        def stt_op(dst, in0, scalar, in1, op0, op1):
            # TensorScalarPtr is not executable on GpSimd (walrus engine check)
            nc.vector.scalar_tensor_tensor(dst, in0, scalar, in1, op0, op1)

        def ctM(fam, part, k1):
            return cs[fam + part][:, k1 * P : (k1 + 1) * P]

        def _accmm(dst_blk, plan):
            """plan: list of (lhsT_ap, rhs_ap); accumulate into one psum, evac to dst_blk."""
            ps = psmm.tile([P, W], F32, tag="ps")
            n = len(plan)
            for i, (l, r) in enumerate(plan):
                nc.tensor.matmul(ps[:], l, r, start=(i == 0), stop=(i == n - 1))
            evac_copy(dst_blk, ps[:])

        def _partial(tag, a, b, op):
            t = work.tile([P, W], F32R, tag=tag)
            tt_op(t[:], a, b, op)
            return t

        def ct_fwd_real(xt, tags):
            """Radix-4x128 DFT along partitions of real x -> perm-order complex tiles."""
            ar = work.tile([P, NT * W], F32R, tag=tags[0])
            ai = work.tile([P, NT * W], F32R, tag=tags[1])
            s02 = _partial("p_s02", _blk(xt, 0), _blk(xt, 2), ALU.add)
            d02 = _partial("p_d02", _blk(xt, 0), _blk(xt, 2), ALU.subtract)
            s13 = _partial("p_s13", _blk(xt, 1), _blk(xt, 3), ALU.add)
            d13 = _partial("p_d13", _blk(xt, 1), _blk(xt, 3), ALU.subtract)
            plans = {
                0: (([("r", s02), ("r", s13)]), ([("i", s02), ("i", s13)])),
                1: (([("r", d02), ("i", d13)]), ([("i", d02), ("rn", d13)])),
                2: (([("r", s02), ("rn", s13)]), ([("i", s02), ("in", s13)])),
                3: (([("r", d02), ("in", d13)]), ([("i", d02), ("r", d13)])),
            }
            for k1 in range(NT):
                pre, pim = plans[k1]
                _accmm(_blk(ar, k1), [(ctM("FWD", v, k1), op[:]) for v, op in pre])
                _accmm(_blk(ai, k1), [(ctM("FWD", v, k1), op[:]) for v, op in pim])
            return ar, ai

        def ct_fwd_cplx(sr, si, tags, out_dt=F32R):
            """Radix-4x128 DFT along partitions of complex input -> perm-order tiles."""
            orr = work.tile([P, NT * W], out_dt, tag=tags[0])
            oii = work.tile([P, NT * W], out_dt, tag=tags[1])
            p02r = _partial("p_1", _blk(sr, 0), _blk(sr, 2), ALU.add)
            p02i = _partial("p_2", _blk(si, 0), _blk(si, 2), ALU.add)
            d02r = _partial("p_3", _blk(sr, 0), _blk(sr, 2), ALU.subtract)
            d02i = _partial("p_4", _blk(si, 0), _blk(si, 2), ALU.subtract)
            p13r = _partial("p_5", _blk(sr, 1), _blk(sr, 3), ALU.add)
            p13i = _partial("p_6", _blk(si, 1), _blk(si, 3), ALU.add)
            d13r = _partial("p_7", _blk(sr, 1), _blk(sr, 3), ALU.subtract)
            d13i = _partial("p_8", _blk(si, 1), _blk(si, 3), ALU.subtract)
            plans = {
                0: ([("r", p02r), ("r", p13r), ("in", p02i), ("in", p13i)],
                    [("i", p02r), ("i", p13r), ("r", p02i), ("r", p13i)]),
                2: ([("r", p02r), ("rn", p13r), ("in", p02i), ("i", p13i)],
                    [("i", p02r), ("in", p13r), ("r", p02i), ("rn", p13i)]),
                1: ([("r", d02r), ("r", d13i), ("in", d02i), ("i", d13r)],
                    [("i", d02r), ("i", d13i), ("r", d02i), ("rn", d13r)]),
                3: ([("r", d02r), ("rn", d13i), ("in", d02i), ("in", d13r)],
                    [("i", d02r), ("in", d13i), ("r", d02i), ("r", d13r)]),
            }
            for k1 in range(NT):
                pre, pim = plans[k1]
                _accmm(_blk(orr, k1), [(ctM("FWD", v, k1), op[:]) for v, op in pre])
                _accmm(_blk(oii, k1), [(ctM("FWD", v, k1), op[:]) for v, op in pim])
            return orr, oii

        def ct_inv(gr, gi, tags, out_dt=F32R):
            """Radix-4x128 IDFT along partitions: perm-order input -> natural tiles.

            Butterfly combines read U/V (then D/E) directly from PSUM —
            no intermediate evacuation copies. U/V pairs are consumed
            before D/E are computed so 4 PSUM banks suffice.
            """
            orr = work.tile([P, NT * W], out_dt, tag=tags[0])
            oii = work.tile([P, NT * W], out_dt, tag=tags[1])

            def s_pair(ja, jb, neg):
                vr = "rn" if neg else "r"
                vi = "i" if neg else "in"
                vii = "in" if neg else "i"
                vir = "rn" if neg else "r"
                pre = [(ctM("INV", "r", ja), _blk(gr, ja)), (ctM("INV", "in", ja), _blk(gi, ja)),
                       (ctM("INV", vr, jb), _blk(gr, jb)), (ctM("INV", vi, jb), _blk(gi, jb))]
                pim = [(ctM("INV", "i", ja), _blk(gr, ja)), (ctM("INV", "r", ja), _blk(gi, ja)),
                       (ctM("INV", vii, jb), _blk(gr, jb)), (ctM("INV", vir, jb), _blk(gi, jb))]
                psr = psmm.tile([P, W], F32, tag="ps")
                psi = psmm.tile([P, W], F32, tag="ps")
                for ps, plan in ((psr, pre), (psi, pim)):
                    for i, (l, r) in enumerate(plan):
                        nc.tensor.matmul(ps[:], l, r, start=(i == 0), stop=(i == 3))
                return psr, psi

            # DVE reads at most one operand from PSUM: evacuate U/D to SBUF,
            # keep V/E in PSUM as the second combine operand (saves 8 copies).
            # U = S0+S2, V = S1+S3 -> x0 = U+V, x2 = U-V
            upr, upi = s_pair(0, 2, False)
            usr = work.tile([P, W], F32, tag="q_ur")
            usi = work.tile([P, W], F32, tag="q_ui")
            evac_copy(usr[:], upr[:])
            evac_copy(usi[:], upi[:])
            vr_, vi_ = s_pair(1, 3, False)
            tt_op(_blk(orr, 0), usr[:], vr_[:], ALU.add)
            tt_op(_blk(oii, 0), usi[:], vi_[:], ALU.add)
            tt_op(_blk(orr, 2), usr[:], vr_[:], ALU.subtract)
            tt_op(_blk(oii, 2), usi[:], vi_[:], ALU.subtract)
            # D = S0-S2, E = S1-S3 -> x1 = D+iE, x3 = D-iE
            dpr, dpi = s_pair(0, 2, True)
            dsr = work.tile([P, W], F32, tag="q_dr")
            dsi = work.tile([P, W], F32, tag="q_di")
            evac_copy(dsr[:], dpr[:])
            evac_copy(dsi[:], dpi[:])
            er, ei = s_pair(1, 3, True)
            tt_op(_blk(orr, 1), dsr[:], ei[:], ALU.subtract)
            tt_op(_blk(oii, 1), dsi[:], er[:], ALU.add)
            tt_op(_blk(orr, 3), dsr[:], ei[:], ALU.add)
            tt_op(_blk(oii, 3), dsi[:], er[:], ALU.subtract)
            return orr, oii

        def transpose_mat(src, tag):
            """dst = src^T for [512,512] f32r matrix in (t p) layout."""
            dst = work.tile([P, NT * W], F32R, tag=tag)
            for jt in range(NT):
                ps = pstp.tile([P, W], F32R)
                for it in range(NT):
                    nc.tensor.transpose(
                        ps[:, it * P : (it + 1) * P],
                        src[:, it * W + jt * P : it * W + jt * P + P],
                        cs["ident"][:],
                    )
                evac_copy(_blk(dst, jt), ps[:])
            return dst

        def fft2T(ch, tags):
            """Returns (Br, Bi) = f^T for channel ch, f32r, perm-order rows/cols."""
            xt = load_x(ch)
            ar, ai = ct_fwd_real(xt, ("m1r", "m1i"))
            tar = transpose_mat(ar, "m2r")
            tai = transpose_mat(ai, "m2i")
            return ct_fwd_cplx(tar, tai, tags)

        # ---- phase A: channel 8 spectrum + cutoff + mask vectors ----------
        b8r, b8i = fft2T(8, ("b8r", "b8i"))
        # persist B8 by copying pool: use persist pool tiles
        # (fft2T wrote into work pool tags b8r/b8i; keep them alive by tagging in work pool
        #  and not reusing those tags later.)

        # mag2 = b8r^2 + b8i^2, f32r
        mag = work.tile([P, NT * W], F32R, tag="mag")
        tmp = work.tile([P, NT * W], F32, tag="gr")
        for t in range(NT):
            nc.vector.tensor_tensor(_blk(tmp, t), _blk(b8r, t), _blk(b8r, t), ALU.mult)
            nc.vector.scalar_tensor_tensor(
                _blk(mag, t), _blk(b8i, t), 1.0, _blk(b8i, t), ALU.mult, ALU.mult
            )
            nc.vector.tensor_tensor(_blk(mag, t), _blk(mag, t), _blk(tmp, t), ALU.add)

        # T1 = Asel^T @ mag  -> [256 (2 tiles), 512]; E = rowsum(T1 * Bsel)
        e_tiles = []
        for mt in range(2):
            ps = psmm.tile([P, W], F32)
            for kt in range(NT):
                nc.tensor.matmul(
                    ps[:], cs["Asel"][:, kt * 256 + mt * P : kt * 256 + mt * P + P],
                    _blk(mag, kt), start=(kt == 0), stop=(kt == NT - 1),
                )
            msk = work.tile([P, W], F32, tag="msk")
            nc.vector.tensor_tensor(msk[:], ps[:], _blk(cs["Bsel"], mt), ALU.mult)
            ev = persist.tile([P, 1], F32, tag=f"e{mt}")
            nc.vector.tensor_reduce(ev[:], msk[:], op=ALU.add, axis=AX.X)
            e_tiles.append(ev)

        # total at e1[127]; move to partition 0, thr = 0.9*total
        mv = persist.tile([1, 1], F32, tag="mv")
        nc.gpsimd.dma_start(mv[:], e_tiles[1][127:128, 0:1])
        thr = persist.tile([1, 1], F32, tag="thr")
        nc.vector.tensor_scalar(thr[:], mv[:], ENERGY, None, ALU.mult)
        psb = pssm.tile([P, 1], F32, tag="sm")
        nc.tensor.matmul(psb[:], cs["ones_row"][:], thr[:], start=True, stop=True)
        thr_bc = persist.tile([P, 1], F32, tag="thr_bc")
        nc.any.tensor_copy(thr_bc[:], psb[:])

        # cnt = sum over c=1..255 of (E < thr)
        nok0 = persist.tile([P, 1], F32, tag="nok0")
        nok1 = persist.tile([P, 1], F32, tag="nok1")
        nc.vector.tensor_scalar(nok0[:], e_tiles[0][:], thr_bc[:], None, ALU.is_lt)
        nc.vector.tensor_scalar(nok1[:], e_tiles[1][:], thr_bc[:], None, ALU.is_lt)
        pcnt = pssm.tile([1, 1], F32, tag="sm")
        nc.tensor.matmul(pcnt[:], nok0[:], cs["ones_col"][:], start=True, stop=False)
        nc.tensor.matmul(pcnt[:], nok1[:127], cs["ones_col"][:127], start=False, stop=True)
        cnt = persist.tile([1, 1], F32, tag="cnt")
        nc.any.tensor_copy(cnt[:], pcnt[:])

        # cval = cnt+1 if cnt < 255 else 5
        aa = persist.tile([1, 1], F32, tag="aa")
        fb = persist.tile([1, 1], F32, tag="fb")
        uu = persist.tile([1, 1], F32, tag="uu")
        cval = persist.tile([1, 1], F32, tag="cval")
        nc.vector.tensor_scalar(aa[:], cnt[:], 1.0, None, ALU.add)
        nc.vector.tensor_scalar(fb[:], cnt[:], 254.5, None, ALU.is_ge)
        nc.vector.tensor_scalar(uu[:], aa[:], 5.0, None, ALU.subtract)
        nc.vector.tensor_tensor(uu[:], uu[:], fb[:], ALU.mult)
        nc.vector.tensor_tensor(cval[:], aa[:], uu[:], ALU.subtract)

        # broadcast cval -> [128,1]; c2 = 512 - c
        psb2 = pssm.tile([P, 1], F32, tag="sm")
        nc.tensor.matmul(psb2[:], cs["ones_row"][:], cval[:], start=True, stop=True)
        c_bc = persist.tile([P, 1], F32, tag="c_bc")
        nc.any.tensor_copy(c_bc[:], psb2[:])
        c2_bc = persist.tile([P, 1], F32, tag="c2_bc")
        nc.vector.tensor_scalar(c2_bc[:], c_bc[:], -1.0, 512.0, ALU.mult, ALU.add)

        # in_c [128, 512], in_r [128, 4]
        in_c = persist.tile([P, W], F32, tag="in_c")
        tmpc = persist.tile([P, W], F32, tag="tmpc")
        nc.vector.tensor_scalar(in_c[:], cs["iota_f"][:], c_bc[:], None, ALU.is_lt)
        nc.vector.tensor_scalar(tmpc[:], cs["iota_f"][:], c2_bc[:], None, ALU.is_ge)
        nc.vector.tensor_tensor(in_c[:], in_c[:], tmpc[:], ALU.max)
        in_r = persist.tile([P, NT], F32, tag="in_r")
        tmpr = persist.tile([P, NT], F32, tag="tmpr")
        nc.vector.tensor_scalar(in_r[:], cs["iota_p"][:], c_bc[:], None, ALU.is_lt)
        nc.vector.tensor_scalar(tmpr[:], cs["iota_p"][:], c2_bc[:], None, ALU.is_ge)
        nc.vector.tensor_tensor(in_r[:], in_r[:], tmpr[:], ALU.max)

        # keep-mask, built once per core: keep[t-block] = 1 - in_r[p,t] * in_c
        # = (in_c * -in_r[p,t]) + 1.  Reuses the mag slot (dead after cutoffs).
        neg_r = persist.tile([P, NT], F32, tag="neg_r")
        nc.vector.tensor_scalar(neg_r[:], in_r[:], -1.0, None, ALU.mult)
        keep = work.tile([P, NT * W], F32R, tag="mag")
        for t in range(NT):
            nc.vector.tensor_scalar(
                _blk(keep, t), in_c[:], neg_r[:, t : t + 1], 1.0, ALU.mult, ALU.add
            )

        # ---- phase B: all channels --------------------------------------
        for ch in range(C):
            if ch == 8:
                br, bi = b8r, b8i
            else:
                br, bi = fft2T(ch, ("m3r", "m3i"))

            # mask: G = B * keep (keep precomputed once in phase A)
            gr = work.tile([P, NT * W], F32R, tag="gr")
            gi = work.tile([P, NT * W], F32R, tag="gi")
            for t in range(NT):
                for src, dstt in ((br, gr), (bi, gi)):
                    tt_op(_blk(dstt, t), _blk(src, t), _blk(keep, t), ALU.mult)

            # inverse: C1 = IDFT_rows(G) ; Y = IDFT_rows(C1^T)
            c1r, c1i = ct_inv(gr, gi, ("m1r", "m1i"))
            tc1r = transpose_mat(c1r, "m2r")
            tc1i = transpose_mat(c1i, "m2i")
            yr, yi = ct_inv(tc1r, tc1i, ("yr", "yi"), out_dt=F32)

            # abs + store
            ot = opool.tile([P, NT * W], F32, tag="ot")
            for t in range(NT):
                nc.vector.tensor_tensor(_blk(yr, t), _blk(yr, t), _blk(yr, t), ALU.mult)
                nc.vector.scalar_tensor_tensor(
                    _blk(ot, t), _blk(yi, t), 1.0, _blk(yi, t), ALU.mult, ALU.mult
                )
                nc.vector.tensor_tensor(_blk(ot, t), _blk(ot, t), _blk(yr, t), ALU.add)
                nc.scalar.activation(_blk(ot, t), _blk(ot, t), ACTF.Sqrt)
            nc.sync.dma_start(
                out_d.ap()[ch].rearrange("(t p) j -> p t j", p=P),
                ot[:].rearrange("p (t j) -> p t j", t=NT),
            )

    nc.compile()
    return nc


# ----------------------------------------------------------------- pjrt runner
_CACHE = {}


def _make_runner():
    """Compile once; returns callable taking full x [8,16,512,512] -> [8,16,512,512]."""
    from jax.sharding import Mesh, PartitionSpec
    from jax.experimental.shard_map import shard_map
    from concourse import bass2jax
    from concourse.bass2jax import _bass_exec_p, install_neuronx_cc_hook, partition_id_tensor

    install_neuronx_cc_hook()
    nc = build_nc()
    consts = _host_constants()

    partition_name = nc.partition_id_tensor.name if nc.partition_id_tensor else None
    in_names = []
    out_names = []
    out_avals = []
    for alloc in nc.m.functions[0].allocations:
        if not isinstance(alloc, mybir.MemoryLocationSet):
            continue
        name = alloc.memorylocations[0].name
        if alloc.kind == "ExternalInput":
            if name != partition_name:
                in_names.append(name)
        elif alloc.kind == "ExternalOutput":
            out_names.append(name)
            out_avals.append(
                jax.core.ShapedArray(tuple(alloc.tensor_shape), mybir.dt.np(alloc.dtype))
            )
    n_params = len(in_names)
    n_outs = len(out_avals)
    all_names = in_names + out_names
    if partition_name is not None:
        all_names = all_names + [partition_name]

    def _body(*args):
        operands = list(args)
        if partition_name is not None:
            operands.append(partition_id_tensor())
        outs = _bass_exec_p.bind(
            *operands,
            out_avals=tuple(out_avals),
            in_names=tuple(all_names),
            out_names=tuple(out_names),
            lowering_input_output_aliases=(),
            sim_require_finite=True,
            sim_require_nnan=True,
            nc=nc,
        )
        return tuple(outs)

    devices = jax.devices()[:NCORES]
    mesh = Mesh(np.asarray(devices), ("core",))
    donate = tuple(range(n_params, n_params + n_outs))
    sharded = jax.jit(
        shard_map(
            _body,
            mesh=mesh,
            in_specs=(PartitionSpec("core"),) * (n_params + n_outs),
            out_specs=(PartitionSpec("core"),) * n_outs,
            check_rep=False,
        ),
        donate_argnums=donate,
        keep_unused=True,
    )

    from jax.sharding import NamedSharding
    import jax.numpy as jnp

    shard = NamedSharding(mesh, PartitionSpec("core"))

    # device-resident constants, uploaded once
    consts_dev = {}
    for name in in_names:
        if name == "x":
            continue
        consts_dev[name] = jax.device_put(
            np.concatenate([consts[name]] * NCORES, axis=0), shard
        )

    import os as _os
    import time as _time
    _dbg = _os.environ.get("KERNEL_DEBUG_TIMING")

    def run(x_full):
        t0 = _time.time()
        per_core_inputs = []
        for name in in_names:
            if name == "x":
                xd = jax.device_put(x_full.reshape(NCORES * C, H, W), shard)
                xd.block_until_ready()
                per_core_inputs.append(xd)
            else:
                per_core_inputs.append(consts_dev[name])
        t1 = _time.time()
        zeros = [
            jax.device_put(
                jnp.zeros((NCORES * a.shape[0], *a.shape[1:]), a.dtype), shard
            )
            for a in out_avals
        ]
        for z in zeros:
            z.block_until_ready()
        t2 = _time.time()
        out_arrs = sharded(*per_core_inputs, *zeros)
        for o in out_arrs:
            o.block_until_ready()
        t3 = _time.time()
        globals()["LAST_EXEC_S"] = t3 - t2
        o = np.asarray(out_arrs[out_names.index("out")])
        t4 = _time.time()
        if _dbg:
            print(f"[timing] h2d_x={t1-t0:.3f}s zeros={t2-t1:.3f}s exec={t3-t2:.3f}s d2h={t4-t3:.3f}s")
        return o.reshape(NCORES, C, H, W)

    return run


def kernel(x):
    x = np.ascontiguousarray(np.asarray(x, dtype=np.float32))
    assert x.shape == (B, C, H, W)
    if "run" not in _CACHE:
        _CACHE["run"] = _make_runner()
    return _CACHE["run"](x).astype(np.float32)


if __name__ == "__main__":
    rng = np.random.default_rng(0)
    x = rng.standard_normal((B, C, H, W), dtype=np.float32)
    y = kernel(x)
    print(y.shape, y.dtype, float(y.mean()))

